# revision 25
# baseline (speedup 1.0000x reference)
"""Trainium2 Bass kernel for nn_BartCrossAttention (B=4, L=1024, D=1024, H=16, HD=64).

v2 sharding: core c -> (batch b = c//2, head-half j = c%2). Each core computes
heads [8j, 8j+8) for ALL 1024 query tokens of its batch, including the K/V/Q
projections restricted to its 512 features, then a PARTIAL out-projection
(contracting only its 512 ctx features). The host sums the two partial outputs
per batch and adds out_bias. No KV-projection duplication, no collectives.

Per-core dataflow (bf16 on every PE input, fp32 PSUM accumulation):
  stage:  kv/hid -> SBUF (2 big DMAs each); weights via gpsimd-issued DMAs
  PE-transpose kv -> kvT [128,8,1024], hid -> hidT
  V = kvT.T @ Wv + vb  -> v65 [tok, head-blocks of 64 | ones col] (ones col
      gives softmax denominators for free in AV row 64)
  K^T = Wk.T @ kvT + kb -> KT [128,4,1024]; Q^T likewise (Wq pre-scaled 1/8)
  per head h (f = h//2, partitions rb=64*(h%2)): per t (kv tile), chunk c:
      S^T = KT_h.T @ qT_h (single 64-contraction matmul); attn = exp(S^T) on
      ACT (psum->bf16); ctx_ps[c] += [V_h|1].T @ attn
  normalization pipelined into the next head's slots: recip of sums row (DVE)
      -> ones-broadcast matmul (PE, 64x512) -> fused evict-multiply into ctxT
  out partial = ctxT.T @ Wo -> bf16 -> DRAM (bias added on host)
"""
import sys

for _p in ("/opt/trn_rl_repo",):
    if _p not in sys.path:
        sys.path.insert(0, _p)

import numpy as np
import ml_dtypes

import concourse.bass as bass
import concourse.mybir as mybir
import concourse.tile as tile
from concourse import bacc
import concourse.bass_utils as bass_utils
from concourse.masks import make_identity

F32 = mybir.dt.float32
F32R = mybir.dt.float32r
BF16 = mybir.dt.bfloat16
NPBF16 = ml_dtypes.bfloat16

P = 128
D = 1024        # model dim
H = 16          # heads (global)
HPC = 8         # heads per core
FPC = 512       # features per core
NCORES = 8
B, LQ, LK = 4, 1024, 1024

_CACHE = {}


def _build_core_program():
    nc = bacc.Bacc("TRN2", target_bir_lowering=False, debug=False,
                   num_devices=NCORES)

    hid_s = nc.dram_tensor("hid_s", [LQ, D], BF16, kind="ExternalInput")
    kv_s = nc.dram_tensor("kv_s", [LK, D], BF16, kind="ExternalInput")
    wq_t = nc.dram_tensor("wq_t", [D, FPC], BF16, kind="ExternalInput")
    wk_t = nc.dram_tensor("wk_t", [D, FPC], BF16, kind="ExternalInput")
    wv_t = nc.dram_tensor("wv_t", [D, FPC], BF16, kind="ExternalInput")
    wo_t = nc.dram_tensor("wo_t", [FPC, D], BF16, kind="ExternalInput")
    qb_d = nc.dram_tensor("qb", [P, 4], F32, kind="ExternalInput")
    kb_d = nc.dram_tensor("kb", [P, 4], F32, kind="ExternalInput")
    vb_d = nc.dram_tensor("vb", [1, FPC], F32, kind="ExternalInput")
    out_s = nc.dram_tensor("out_s", [LQ, D], BF16, kind="ExternalOutput")

    Exp = mybir.ActivationFunctionType.Exp
    Ident = mybir.ActivationFunctionType.Identity
    add = mybir.AluOpType.add
    mult = mybir.AluOpType.mult

    with tile.TileContext(nc) as tc:
        with (
            tc.tile_pool(name="setup", bufs=1) as setup,
            tc.tile_pool(name="big", bufs=1) as big,
        ):
            # ---- staging tiles + all input DMAs up front, spread across
            # issue engines so no single queue serializes the prologue ----
            kv_nat = big.tile([P, 8, D], BF16, tag="kv_nat")
            hid_nat = big.tile([P, 8, D], BF16, tag="hid_nat")
            wk = setup.tile([P, 8, FPC], BF16, tag="wk")
            wq = setup.tile([P, 8, FPC], BF16, tag="wq")
            wv = setup.tile([P, 8, FPC], BF16, tag="wv")
            wo = setup.tile([P, 4, D], BF16, tag="wo")

            kv_r = kv_s.ap().rearrange("(tt p) d -> p tt d", p=P)
            hid_r = hid_s.ap().rearrange("(tt p) d -> p tt d", p=P)
            for tt in range(8):
                nc.sync.dma_start(kv_nat[:, tt:tt + 1, :],
                                  kv_r[:, tt:tt + 1, :])
            nc.scalar.dma_start(hid_nat[:, 0:4, :], hid_r[:, 0:4, :])
            nc.scalar.dma_start(hid_nat[:, 4:8, :], hid_r[:, 4:8, :])
            nc.gpsimd.dma_start(
                wv[:], wv_t.ap().rearrange("(dd p) o -> p dd o", p=P))
            nc.gpsimd.dma_start(
                wk[:], wk_t.ap().rearrange("(dd p) o -> p dd o", p=P))
            nc.gpsimd.dma_start(
                wq[:], wq_t.ap().rearrange("(dd p) o -> p dd o", p=P))
            nc.gpsimd.dma_start(
                wo[:], wo_t.ap().rearrange("(dd p) o -> p dd o", p=P))

            qb_sb = setup.tile([P, 4], F32, tag="qb")
            nc.gpsimd.dma_start(qb_sb[:], qb_d.ap())
            kb_sb = setup.tile([P, 4], F32, tag="kb")
            nc.gpsimd.dma_start(kb_sb[:], kb_d.ap())
            vb_row = setup.tile([1, FPC], F32, tag="vb_row")
            nc.gpsimd.dma_start(vb_row[:], vb_d.ap())

            # ---- small setup ----
            identF = setup.tile([P, P], F32, tag="identF")
            make_identity(nc, identF[:])
            ident = setup.tile([P, P], BF16, tag="ident")
            nc.vector.tensor_copy(ident[:], identF[:])
            vbB = setup.tile([P, FPC], F32, tag="vbB")
            nc.gpsimd.partition_broadcast(vbB[:], vb_row[:])

            # ---- persistent big tiles ----
            kvT = big.tile([P, 8, LK], BF16, tag="kvT")    # kv^T [1024,1024]
            hidT = big.tile([P, 8, LQ], BF16, tag="hidT")  # hid^T
            KT = big.tile([P, 4, LK], BF16, tag="KT")      # K^T [512,1024]
            qT = big.tile([P, 4, LQ], BF16, tag="qT")      # Q^T [512,1024]
            v65 = big.tile([P, 8, HPC * 65], BF16, tag="v65")
            ctxT = big.tile([P, 4, LQ], BF16, tag="ctxT")  # ctx^T [512,1024]

            # ones column (col 64 of each head block) for denominators
            onesF = setup.tile([P, 64], F32, tag="onesF")
            nc.gpsimd.memset(onesF[:], 1.0)
            nc.vector.tensor_copy(
                v65[:].rearrange("p t (h x) -> p t h x", x=65)[:, :, :, 64:65],
                onesF[:].rearrange("p (t h x) -> p t h x", t=8, h=8))

            # ---- HAM warm-up: ~24 full-array matmuls on junk data while
            # the first kv chunks are still in flight. Triggers the PE
            # activity monitor to 8/8 (2.4 GHz) before real work arrives ----
            with tc.tile_pool(name="warm", bufs=2, space="PSUM") as warmp:
                for i in range(24):
                    wps = warmp.tile([P, P], F32, tag="warm", name=f"wm{i}")
                    nc.tensor.matmul(wps[:], ident[:], ident[:],
                                     start=True, stop=True)

            # ---- transposes: nat [p, tt, d] -> T [p, dd, tt*128] ----
            def transpose_in(dst, nat, pool):
                # 4 transposed 128x128 tiles packed per psum tile, one
                # strided eviction each (alternating scalar/vector)
                for tt in range(8):
                    for g in range(2):
                        tp = pool.tile([P, 512], BF16, tag="tp",
                                       name=f"tp{tt}_{g}")
                        for dl in range(4):
                            di = g * 4 + dl
                            nc.tensor.transpose(
                                tp[:, dl * P:(dl + 1) * P],
                                nat[:, tt, di * P:(di + 1) * P],
                                ident[:],
                            )
                        d_ap = dst[:, g * 4:(g + 1) * 4, tt * P:(tt + 1) * P]
                        s_ap = tp[:].rearrange("p (d x) -> p d x", d=4)
                        if g == 0:
                            nc.scalar.activation(d_ap, s_ap, Ident)
                        else:
                            nc.vector.tensor_copy(d_ap, s_ap)

            if True:
                # ---- V projection: v65[:, ti, h*65:h*65+64] ----
                def emit_vproj(ti, pool):
                    pv = pool.tile([P, FPC], F32, tag="pp", name=f"pv{ti}")
                    for dd in range(8):
                        nc.tensor.matmul(
                            pv[:],
                            kvT[:, dd, ti * P:(ti + 1) * P],
                            wv[:, dd, :],
                            start=(dd == 0), stop=(dd == 7),
                        )
                    dst = v65[:].rearrange("p t (h x) -> p t h x", x=65)[
                        :, ti, :, 0:64]
                    nc.vector.tensor_tensor(dst, pv[:], vbB[:], add)

                # ---- K^T / Q^T projections (feature tile f, token chunk ck)
                def emit_kproj(f, ck, pool):
                    pk = pool.tile([P, FPC], F32, tag="pp", name=f"pk{f}_{ck}")
                    for dd in range(8):
                        nc.tensor.matmul(
                            pk[:],
                            wk[:, dd, f * P:(f + 1) * P],
                            kvT[:, dd, ck * 512:(ck + 1) * 512],
                            start=(dd == 0), stop=(dd == 7),
                        )
                    nc.vector.tensor_scalar(
                        KT[:, f, ck * 512:(ck + 1) * 512], pk[:],
                        kb_sb[:, f:f + 1], None, add)

                def emit_qproj(f, ck, pool):
                    pq = pool.tile([P, FPC], F32, tag="pp", name=f"pq{f}_{ck}")
                    for dd in range(8):
                        nc.tensor.matmul(
                            pq[:],
                            wq[:, dd, f * P:(f + 1) * P],
                            hidT[:, dd, ck * 512:(ck + 1) * 512],
                            start=(dd == 0), stop=(dd == 7),
                        )
                    nc.vector.tensor_scalar(
                        qT[:, f, ck * 512:(ck + 1) * 512], pq[:],
                        qb_sb[:, f:f + 1], None, add)

                with tc.tile_pool(name="psmmA", bufs=3,
                                  space="PSUM") as psmmA:
                    # kv transposes, then V/K0 (cover the hid DMA window),
                    # then hid transposes, then Q0
                    transpose_in(kvT, kv_nat, psmmA)
                    for ti in range(8):
                        emit_vproj(ti, psmmA)
                    for ck in range(2):
                        emit_kproj(0, ck, psmmA)
                    transpose_in(hidT, hid_nat, psmmA)
                    for ck in range(2):
                        emit_qproj(0, ck, psmmA)

                # ---- attention main loop ----
                with (
                    tc.tile_pool(name="scp", bufs=2, space="PSUM") as scp,
                    tc.tile_pool(name="ctxp", bufs=3, space="PSUM") as ctxp,
                    tc.tile_pool(name="psmmB", bufs=1, space="PSUM") as psmmB,
                    tc.tile_pool(name="bcbp", bufs=2) as bcbp,
                    tc.tile_pool(name="atp", bufs=3) as atp,
                    tc.tile_pool(name="rcpp", bufs=2) as rcpp,
                ):
                    def emit_norm_recips(h, ctx_pair):
                        # sums rows: psum -> sbuf stage, then fast reciprocal
                        rcps = []
                        for c in range(2):
                            stg = rcpp.tile([1, FPC], F32, tag=f"stg{c}",
                                            name=f"stg{h}_{c}")
                            nc.vector.tensor_copy(stg[:],
                                                  ctx_pair[c][64:65, :])
                            rcp = rcpp.tile([1, FPC], F32, tag=f"rcp{c}",
                                            name=f"rcp{h}_{c}")
                            nc.vector.reciprocal_approx_fast(rcp[:], stg[:])
                            rcps.append(rcp)
                        return rcps

                    def emit_norm_bc(h, rcps):
                        # broadcast each recip row to 64 partitions (gpsimd)
                        bcs = []
                        for c in range(2):
                            bcb = bcbp.tile([64, FPC], F32, tag=f"bcb{c}",
                                            name=f"bcb{h}_{c}")
                            nc.gpsimd.partition_broadcast(bcb[:], rcps[c][:])
                            bcs.append(bcb)
                        return bcs

                    def emit_norm_mult(h, ctx_pair, bcs):
                        f, rb = h // 2, 64 * (h % 2)
                        for c in range(2):
                            nc.vector.tensor_tensor(
                                ctxT[rb:rb + 64, f, c * 512:(c + 1) * 512],
                                ctx_pair[c][0:64, :], bcs[c][:], mult)

                    prev = None  # head pending normalization
                    for h in range(HPC):
                        f, rb = h // 2, 64 * (h % 2)
                        nxt_f = f + 1
                        ctx_pair = [ctxp.tile([65, FPC], F32, tag="ctx",
                                              name=f"ctx{h}_{c}")
                                    for c in range(2)]
                        at_prev = None
                        for t in range(8):
                            sc = scp.tile([P, 2 * FPC], F32, tag="sc",
                                          name=f"sc{h}_{t}")
                            for c in range(2):
                                nc.tensor.matmul(
                                    sc[:, c * FPC:(c + 1) * FPC],
                                    KT[rb:rb + 64, f, t * P:(t + 1) * P],
                                    qT[rb:rb + 64, f, c * 512:(c + 1) * 512],
                                    start=True, stop=True,
                                )
                            # pipelined normalization of the previous head
                            if prev is not None and t == 0:
                                ph, pctx = prev
                                rcps = emit_norm_recips(ph, pctx)
                                bcs = emit_norm_bc(ph, rcps)
                                emit_norm_mult(ph, pctx, bcs)
                                prev = None
                            # one 1024-wide exp per t (full ACT column rate)
                            at = atp.tile([P, 2 * FPC], BF16, tag="at",
                                          name=f"at{h}_{t}")
                            nc.scalar.activation(at[:], sc[:], Exp)
                            # AV lags one t: PE never waits on a fresh exp
                            if at_prev is not None:
                                for c in range(2):
                                    nc.tensor.matmul(
                                        ctx_pair[c][:],
                                        v65[:, t - 1, h * 65:(h + 1) * 65],
                                        at_prev[:, c * FPC:(c + 1) * FPC],
                                        start=(t == 1), stop=False,
                                    )
                            at_prev = at
                            # full-array projection filler: keeps the PE HAM
                            # activity monitor asserting warm (2.4 GHz) during
                            # the half-array attention matmuls
                            if nxt_f < 4:
                                if h % 2 == 0:
                                    if t == 2:
                                        emit_kproj(nxt_f, 0, psmmB)
                                    elif t == 5:
                                        emit_kproj(nxt_f, 1, psmmB)
                                else:
                                    if t == 2:
                                        emit_qproj(nxt_f, 0, psmmB)
                                    elif t == 5:
                                        emit_qproj(nxt_f, 1, psmmB)
                        for c in range(2):
                            nc.tensor.matmul(
                                ctx_pair[c][:],
                                v65[:, 7, h * 65:(h + 1) * 65],
                                at_prev[:, c * FPC:(c + 1) * FPC],
                                start=False, stop=True,
                            )
                        prev = (h, ctx_pair)
                    # final head's normalization
                    ph, pctx = prev
                    rcps = emit_norm_recips(ph, pctx)
                    bcs = emit_norm_bc(ph, rcps)
                    emit_norm_mult(ph, pctx, bcs)

            # ---- epilogue: partial out projection (no bias; host adds) ----
            with (
                tc.tile_pool(name="pop", bufs=2, space="PSUM") as pop,
                tc.tile_pool(name="outp", bufs=2) as outp,
            ):
                for m in range(8):
                    ot = outp.tile([P, D], BF16, tag="ot", name=f"ot{m}")
                    for half in range(2):
                        po = pop.tile([P, FPC], F32, tag="po",
                                      name=f"po{m}_{half}")
                        for fj in range(4):
                            nc.tensor.matmul(
                                po[:],
                                ctxT[:, fj, m * P:(m + 1) * P],
                                wo[:, fj, half * 512:(half + 1) * 512],
                                start=(fj == 0), stop=(fj == 3),
                            )
                        if half == 0:
                            nc.scalar.activation(
                                ot[:, half * 512:(half + 1) * 512], po[:],
                                Ident)
                        else:
                            nc.vector.tensor_copy(
                                ot[:, half * 512:(half + 1) * 512], po[:])
                    eng = nc.sync if m % 2 == 0 else nc.gpsimd
                    eng.dma_start(
                        out_s.ap().rearrange("(mm p) d -> p mm d", p=P)[
                            :, m, :],
                        ot[:])

    nc.compile()
    return nc


def _prep_inputs(hidden_states, key_value_states, q_weight, q_bias,
                 kv_weight, kv_bias, out_weight, out_bias):
    f32 = np.float32
    hid = np.asarray(hidden_states, f32).reshape(B, LQ, D).astype(NPBF16)
    kv = np.asarray(key_value_states, f32).reshape(B, LK, D).astype(NPBF16)
    scale = f32(1.0 / 8.0)

    # de-interleave kv rows: row e <-> (h=e//128, j=(e%128)//64, d=e%64)
    e = np.arange(2 * D)
    kmask = (e % 128) < 64
    kidx, vidx = e[kmask], e[~kmask]
    kvw = np.asarray(kv_weight, f32)
    kvb = np.asarray(kv_bias, f32)

    wq_full = (np.asarray(q_weight, f32) * scale).T      # [D, D] d x feat
    wk_full = kvw[kidx].T                                # [D, D]
    wv_full = kvw[vidx].T
    wo_full = np.asarray(out_weight, f32).T              # [D, D] feat x out
    qb_full = np.asarray(q_bias, f32) * scale
    kb_full = kvb[kidx]
    vb_full = kvb[vidx]

    jmaps = []
    for j in range(2):
        s = slice(j * FPC, (j + 1) * FPC)
        jmaps.append({
            "wq_t": np.ascontiguousarray(wq_full[:, s].astype(NPBF16)),
            "wk_t": np.ascontiguousarray(wk_full[:, s].astype(NPBF16)),
            "wv_t": np.ascontiguousarray(wv_full[:, s].astype(NPBF16)),
            "wo_t": np.ascontiguousarray(wo_full[s, :].astype(NPBF16)),
            "qb": np.ascontiguousarray(qb_full[s].reshape(4, P).T),
            "kb": np.ascontiguousarray(kb_full[s].reshape(4, P).T),
            "vb": np.ascontiguousarray(vb_full[s].reshape(1, FPC)),
        })
    in_maps = []
    for c in range(NCORES):
        b, j = c // 2, c % 2
        m = dict(jmaps[j])
        m["hid_s"] = np.ascontiguousarray(hid[b])
        m["kv_s"] = np.ascontiguousarray(kv[b])
        in_maps.append(m)
    return in_maps


def kernel(hidden_states, key_value_states, q_weight, q_bias,
           kv_weight, kv_bias, out_weight, out_bias, _trace=False):
    if "nc" not in _CACHE:
        _CACHE["nc"] = _build_core_program()
    nc = _CACHE["nc"]
    in_maps = _prep_inputs(hidden_states, key_value_states, q_weight, q_bias,
                           kv_weight, kv_bias, out_weight, out_bias)
    res = bass_utils.run_bass_kernel_spmd(
        nc, in_maps, core_ids=list(range(NCORES)), trace=_trace)
    _CACHE["last_result"] = res
    ob = np.asarray(out_bias, np.float32)
    out = np.empty((B, LQ, D), np.float32)
    for b in range(B):
        p0 = np.asarray(res.results[2 * b]["out_s"], np.float32)
        p1 = np.asarray(res.results[2 * b + 1]["out_s"], np.float32)
        out[b] = p0 + p1 + ob
    return out


# revision 26
# speedup vs baseline: 1.0022x; 1.0022x over previous
"""Trainium2 Bass kernel for nn_BartCrossAttention (B=4, L=1024, D=1024, H=16, HD=64).

v2 sharding: core c -> (batch b = c//2, head-half j = c%2). Each core computes
heads [8j, 8j+8) for ALL 1024 query tokens of its batch, including the K/V/Q
projections restricted to its 512 features, then a PARTIAL out-projection
(contracting only its 512 ctx features). The host sums the two partial outputs
per batch and adds out_bias. No KV-projection duplication, no collectives.

Per-core dataflow (bf16 on every PE input, fp32 PSUM accumulation):
  stage:  kv/hid -> SBUF (2 big DMAs each); weights via gpsimd-issued DMAs
  PE-transpose kv -> kvT [128,8,1024], hid -> hidT
  V = kvT.T @ Wv + vb  -> v65 [tok, head-blocks of 64 | ones col] (ones col
      gives softmax denominators for free in AV row 64)
  K^T = Wk.T @ kvT + kb -> KT [128,4,1024]; Q^T likewise (Wq pre-scaled 1/8)
  per head h (f = h//2, partitions rb=64*(h%2)): per t (kv tile), chunk c:
      S^T = KT_h.T @ qT_h (single 64-contraction matmul); attn = exp(S^T) on
      ACT (psum->bf16); ctx_ps[c] += [V_h|1].T @ attn
  normalization pipelined into the next head's slots: recip of sums row (DVE)
      -> ones-broadcast matmul (PE, 64x512) -> fused evict-multiply into ctxT
  out partial = ctxT.T @ Wo -> bf16 -> DRAM (bias added on host)
"""
import sys

for _p in ("/opt/trn_rl_repo",):
    if _p not in sys.path:
        sys.path.insert(0, _p)

import numpy as np
import ml_dtypes

import concourse.bass as bass
import concourse.mybir as mybir
import concourse.tile as tile
from concourse import bacc
import concourse.bass_utils as bass_utils
from concourse.masks import make_identity

F32 = mybir.dt.float32
F32R = mybir.dt.float32r
BF16 = mybir.dt.bfloat16
NPBF16 = ml_dtypes.bfloat16

P = 128
D = 1024        # model dim
H = 16          # heads (global)
HPC = 8         # heads per core
FPC = 512       # features per core
NCORES = 8
B, LQ, LK = 4, 1024, 1024

_CACHE = {}


def _build_core_program():
    nc = bacc.Bacc("TRN2", target_bir_lowering=False, debug=False,
                   num_devices=NCORES)

    hid_s = nc.dram_tensor("hid_s", [LQ, D], BF16, kind="ExternalInput")
    kv_s = nc.dram_tensor("kv_s", [LK, D], BF16, kind="ExternalInput")
    wq_t = nc.dram_tensor("wq_t", [D, FPC], BF16, kind="ExternalInput")
    wk_t = nc.dram_tensor("wk_t", [D, FPC], BF16, kind="ExternalInput")
    wv_t = nc.dram_tensor("wv_t", [D, FPC], BF16, kind="ExternalInput")
    wo_t = nc.dram_tensor("wo_t", [FPC, D], BF16, kind="ExternalInput")
    qb_d = nc.dram_tensor("qb", [P, 4], F32, kind="ExternalInput")
    kb_d = nc.dram_tensor("kb", [P, 4], F32, kind="ExternalInput")
    vb_d = nc.dram_tensor("vb", [1, FPC], F32, kind="ExternalInput")
    out_s = nc.dram_tensor("out_s", [LQ, D], BF16, kind="ExternalOutput")

    Exp = mybir.ActivationFunctionType.Exp
    Ident = mybir.ActivationFunctionType.Identity
    add = mybir.AluOpType.add
    mult = mybir.AluOpType.mult

    with tile.TileContext(nc) as tc:
        with (
            tc.tile_pool(name="setup", bufs=1) as setup,
            tc.tile_pool(name="big", bufs=1) as big,
        ):
            # ---- staging tiles + all input DMAs up front, spread across
            # issue engines so no single queue serializes the prologue ----
            kv_nat = big.tile([P, 8, D], BF16, tag="kv_nat")
            hid_nat = big.tile([P, 8, D], BF16, tag="hid_nat")
            wk = setup.tile([P, 8, FPC], BF16, tag="wk")
            wq = setup.tile([P, 8, FPC], BF16, tag="wq")
            wv = setup.tile([P, 8, FPC], BF16, tag="wv")
            wo = setup.tile([P, 4, D], BF16, tag="wo")

            # identity first: gpsimd builds it before issuing its DMAs so
            # the PE warm-up can start immediately
            identF = setup.tile([P, P], F32, tag="identF")
            make_identity(nc, identF[:])
            ident = setup.tile([P, P], BF16, tag="ident")
            nc.vector.tensor_copy(ident[:], identF[:])

            kv_r = kv_s.ap().rearrange("(tt p) d -> p tt d", p=P)
            hid_r = hid_s.ap().rearrange("(tt p) d -> p tt d", p=P)
            for tt in range(8):
                nc.sync.dma_start(kv_nat[:, tt:tt + 1, :],
                                  kv_r[:, tt:tt + 1, :])
            nc.scalar.dma_start(hid_nat[:, 0:4, :], hid_r[:, 0:4, :])
            nc.scalar.dma_start(hid_nat[:, 4:8, :], hid_r[:, 4:8, :])
            nc.gpsimd.dma_start(
                wv[:], wv_t.ap().rearrange("(dd p) o -> p dd o", p=P))
            nc.gpsimd.dma_start(
                wk[:], wk_t.ap().rearrange("(dd p) o -> p dd o", p=P))
            nc.gpsimd.dma_start(
                wq[:], wq_t.ap().rearrange("(dd p) o -> p dd o", p=P))
            nc.gpsimd.dma_start(
                wo[:], wo_t.ap().rearrange("(dd p) o -> p dd o", p=P))

            qb_sb = setup.tile([P, 4], F32, tag="qb")
            nc.gpsimd.dma_start(qb_sb[:], qb_d.ap())
            kb_sb = setup.tile([P, 4], F32, tag="kb")
            nc.gpsimd.dma_start(kb_sb[:], kb_d.ap())
            vb_row = setup.tile([1, FPC], F32, tag="vb_row")
            nc.gpsimd.dma_start(vb_row[:], vb_d.ap())

            # ---- small setup ----
            vbB = setup.tile([P, FPC], F32, tag="vbB")
            nc.gpsimd.partition_broadcast(vbB[:], vb_row[:])

            # ---- persistent big tiles ----
            kvT = big.tile([P, 8, LK], BF16, tag="kvT")    # kv^T [1024,1024]
            hidT = big.tile([P, 8, LQ], BF16, tag="hidT")  # hid^T
            KT = big.tile([P, 4, LK], BF16, tag="KT")      # K^T [512,1024]
            qT = big.tile([P, 4, LQ], BF16, tag="qT")      # Q^T [512,1024]
            v65 = big.tile([P, 8, HPC * 65], BF16, tag="v65")
            ctxT = big.tile([P, 4, LQ], BF16, tag="ctxT")  # ctx^T [512,1024]

            # ones column (col 64 of each head block) for denominators
            onesF = setup.tile([P, 64], F32, tag="onesF")
            nc.gpsimd.memset(onesF[:], 1.0)
            nc.vector.tensor_copy(
                v65[:].rearrange("p t (h x) -> p t h x", x=65)[:, :, :, 64:65],
                onesF[:].rearrange("p (t h x) -> p t h x", t=8, h=8))

            # ---- HAM warm-up: ~24 full-array matmuls on junk data while
            # the first kv chunks are still in flight. Triggers the PE
            # activity monitor to 8/8 (2.4 GHz) before real work arrives ----
            with tc.tile_pool(name="warm", bufs=1, space="PSUM") as warmp:
                wps = warmp.tile([P, P], F32, tag="warm")
                for i in range(40):
                    nc.tensor.matmul(wps[:], ident[:], ident[:],
                                     start=(i == 0), stop=(i == 39))

            # ---- transposes: nat [p, tt, d] -> T [p, dd, tt*128] ----
            def transpose_in(dst, nat, pool):
                # 4 transposed 128x128 tiles packed per psum tile, one
                # strided eviction each (alternating scalar/vector)
                for tt in range(8):
                    for g in range(2):
                        tp = pool.tile([P, 512], BF16, tag="tp",
                                       name=f"tp{tt}_{g}")
                        for dl in range(4):
                            di = g * 4 + dl
                            nc.tensor.transpose(
                                tp[:, dl * P:(dl + 1) * P],
                                nat[:, tt, di * P:(di + 1) * P],
                                ident[:],
                            )
                        d_ap = dst[:, g * 4:(g + 1) * 4, tt * P:(tt + 1) * P]
                        s_ap = tp[:].rearrange("p (d x) -> p d x", d=4)
                        if g == 0:
                            nc.scalar.activation(d_ap, s_ap, Ident)
                        else:
                            nc.vector.tensor_copy(d_ap, s_ap)

            if True:
                # ---- V projection: v65[:, ti, h*65:h*65+64] ----
                def emit_vproj(ti, pool):
                    pv = pool.tile([P, FPC], F32, tag="pp", name=f"pv{ti}")
                    for dd in range(8):
                        nc.tensor.matmul(
                            pv[:],
                            kvT[:, dd, ti * P:(ti + 1) * P],
                            wv[:, dd, :],
                            start=(dd == 0), stop=(dd == 7),
                        )
                    dst = v65[:].rearrange("p t (h x) -> p t h x", x=65)[
                        :, ti, :, 0:64]
                    nc.vector.tensor_tensor(dst, pv[:], vbB[:], add)

                # ---- K^T / Q^T projections (feature tile f, token chunk ck)
                def emit_kproj(f, ck, pool):
                    pk = pool.tile([P, FPC], F32, tag="pp", name=f"pk{f}_{ck}")
                    for dd in range(8):
                        nc.tensor.matmul(
                            pk[:],
                            wk[:, dd, f * P:(f + 1) * P],
                            kvT[:, dd, ck * 512:(ck + 1) * 512],
                            start=(dd == 0), stop=(dd == 7),
                        )
                    nc.vector.tensor_scalar(
                        KT[:, f, ck * 512:(ck + 1) * 512], pk[:],
                        kb_sb[:, f:f + 1], None, add)

                def emit_qproj(f, ck, pool):
                    pq = pool.tile([P, FPC], F32, tag="pp", name=f"pq{f}_{ck}")
                    for dd in range(8):
                        nc.tensor.matmul(
                            pq[:],
                            wq[:, dd, f * P:(f + 1) * P],
                            hidT[:, dd, ck * 512:(ck + 1) * 512],
                            start=(dd == 0), stop=(dd == 7),
                        )
                    nc.vector.tensor_scalar(
                        qT[:, f, ck * 512:(ck + 1) * 512], pq[:],
                        qb_sb[:, f:f + 1], None, add)

                with tc.tile_pool(name="psmmA", bufs=3,
                                  space="PSUM") as psmmA:
                    # kv transposes, then V/K0 (cover the hid DMA window),
                    # then hid transposes, then Q0
                    transpose_in(kvT, kv_nat, psmmA)
                    for ti in range(8):
                        emit_vproj(ti, psmmA)
                    for ck in range(2):
                        emit_kproj(0, ck, psmmA)
                    transpose_in(hidT, hid_nat, psmmA)
                    for ck in range(2):
                        emit_qproj(0, ck, psmmA)

                # ---- attention main loop ----
                with (
                    tc.tile_pool(name="scp", bufs=2, space="PSUM") as scp,
                    tc.tile_pool(name="ctxp", bufs=3, space="PSUM") as ctxp,
                    tc.tile_pool(name="psmmB", bufs=1, space="PSUM") as psmmB,
                    tc.tile_pool(name="bcbp", bufs=2) as bcbp,
                    tc.tile_pool(name="atp", bufs=3) as atp,
                    tc.tile_pool(name="rcpp", bufs=2) as rcpp,
                ):
                    def emit_norm_recips(h, ctx_pair):
                        # sums rows: psum -> sbuf stage, then fast reciprocal
                        rcps = []
                        for c in range(2):
                            stg = rcpp.tile([1, FPC], F32, tag=f"stg{c}",
                                            name=f"stg{h}_{c}")
                            nc.vector.tensor_copy(stg[:],
                                                  ctx_pair[c][64:65, :])
                            rcp = rcpp.tile([1, FPC], F32, tag=f"rcp{c}",
                                            name=f"rcp{h}_{c}")
                            nc.vector.reciprocal_approx_fast(rcp[:], stg[:])
                            rcps.append(rcp)
                        return rcps

                    def emit_norm_bc(h, rcps):
                        # broadcast each recip row to 64 partitions (gpsimd)
                        bcs = []
                        for c in range(2):
                            bcb = bcbp.tile([64, FPC], F32, tag=f"bcb{c}",
                                            name=f"bcb{h}_{c}")
                            nc.gpsimd.partition_broadcast(bcb[:], rcps[c][:])
                            bcs.append(bcb)
                        return bcs

                    def emit_norm_mult(h, ctx_pair, bcs):
                        f, rb = h // 2, 64 * (h % 2)
                        for c in range(2):
                            nc.vector.tensor_tensor(
                                ctxT[rb:rb + 64, f, c * 512:(c + 1) * 512],
                                ctx_pair[c][0:64, :], bcs[c][:], mult)

                    prev = None  # head pending normalization
                    for h in range(HPC):
                        f, rb = h // 2, 64 * (h % 2)
                        nxt_f = f + 1
                        ctx_pair = [ctxp.tile([65, FPC], F32, tag="ctx",
                                              name=f"ctx{h}_{c}")
                                    for c in range(2)]
                        at_prev = None
                        for t in range(8):
                            sc = scp.tile([P, 2 * FPC], F32, tag="sc",
                                          name=f"sc{h}_{t}")
                            for c in range(2):
                                nc.tensor.matmul(
                                    sc[:, c * FPC:(c + 1) * FPC],
                                    KT[rb:rb + 64, f, t * P:(t + 1) * P],
                                    qT[rb:rb + 64, f, c * 512:(c + 1) * 512],
                                    start=True, stop=True,
                                )
                            # pipelined normalization of the previous head
                            if prev is not None and t == 0:
                                ph, pctx = prev
                                rcps = emit_norm_recips(ph, pctx)
                                bcs = emit_norm_bc(ph, rcps)
                                emit_norm_mult(ph, pctx, bcs)
                                prev = None
                            # one 1024-wide exp per t (full ACT column rate)
                            at = atp.tile([P, 2 * FPC], BF16, tag="at",
                                          name=f"at{h}_{t}")
                            nc.scalar.activation(at[:], sc[:], Exp)
                            # AV lags one t: PE never waits on a fresh exp
                            if at_prev is not None:
                                for c in range(2):
                                    nc.tensor.matmul(
                                        ctx_pair[c][:],
                                        v65[:, t - 1, h * 65:(h + 1) * 65],
                                        at_prev[:, c * FPC:(c + 1) * FPC],
                                        start=(t == 1), stop=False,
                                    )
                            at_prev = at
                            # full-array projection filler: keeps the PE HAM
                            # activity monitor asserting warm (2.4 GHz) during
                            # the half-array attention matmuls
                            if nxt_f < 4:
                                if h % 2 == 0:
                                    if t == 2:
                                        emit_kproj(nxt_f, 0, psmmB)
                                    elif t == 5:
                                        emit_kproj(nxt_f, 1, psmmB)
                                else:
                                    if t == 2:
                                        emit_qproj(nxt_f, 0, psmmB)
                                    elif t == 5:
                                        emit_qproj(nxt_f, 1, psmmB)
                        for c in range(2):
                            nc.tensor.matmul(
                                ctx_pair[c][:],
                                v65[:, 7, h * 65:(h + 1) * 65],
                                at_prev[:, c * FPC:(c + 1) * FPC],
                                start=False, stop=True,
                            )
                        prev = (h, ctx_pair)
                    # final head's normalization
                    ph, pctx = prev
                    rcps = emit_norm_recips(ph, pctx)
                    bcs = emit_norm_bc(ph, rcps)
                    emit_norm_mult(ph, pctx, bcs)

            # ---- epilogue: partial out projection (no bias; host adds) ----
            with (
                tc.tile_pool(name="pop", bufs=2, space="PSUM") as pop,
                tc.tile_pool(name="outp", bufs=2) as outp,
            ):
                for m in range(8):
                    ot = outp.tile([P, D], BF16, tag="ot", name=f"ot{m}")
                    for half in range(2):
                        po = pop.tile([P, FPC], F32, tag="po",
                                      name=f"po{m}_{half}")
                        for fj in range(4):
                            nc.tensor.matmul(
                                po[:],
                                ctxT[:, fj, m * P:(m + 1) * P],
                                wo[:, fj, half * 512:(half + 1) * 512],
                                start=(fj == 0), stop=(fj == 3),
                            )
                        if half == 0:
                            nc.scalar.activation(
                                ot[:, half * 512:(half + 1) * 512], po[:],
                                Ident)
                        else:
                            nc.vector.tensor_copy(
                                ot[:, half * 512:(half + 1) * 512], po[:])
                    eng = nc.sync if m % 2 == 0 else nc.gpsimd
                    eng.dma_start(
                        out_s.ap().rearrange("(mm p) d -> p mm d", p=P)[
                            :, m, :],
                        ot[:])

    nc.compile()
    return nc


def _prep_inputs(hidden_states, key_value_states, q_weight, q_bias,
                 kv_weight, kv_bias, out_weight, out_bias):
    f32 = np.float32
    hid = np.asarray(hidden_states, f32).reshape(B, LQ, D).astype(NPBF16)
    kv = np.asarray(key_value_states, f32).reshape(B, LK, D).astype(NPBF16)
    scale = f32(1.0 / 8.0)

    # de-interleave kv rows: row e <-> (h=e//128, j=(e%128)//64, d=e%64)
    e = np.arange(2 * D)
    kmask = (e % 128) < 64
    kidx, vidx = e[kmask], e[~kmask]
    kvw = np.asarray(kv_weight, f32)
    kvb = np.asarray(kv_bias, f32)

    wq_full = (np.asarray(q_weight, f32) * scale).T      # [D, D] d x feat
    wk_full = kvw[kidx].T                                # [D, D]
    wv_full = kvw[vidx].T
    wo_full = np.asarray(out_weight, f32).T              # [D, D] feat x out
    qb_full = np.asarray(q_bias, f32) * scale
    kb_full = kvb[kidx]
    vb_full = kvb[vidx]

    jmaps = []
    for j in range(2):
        s = slice(j * FPC, (j + 1) * FPC)
        jmaps.append({
            "wq_t": np.ascontiguousarray(wq_full[:, s].astype(NPBF16)),
            "wk_t": np.ascontiguousarray(wk_full[:, s].astype(NPBF16)),
            "wv_t": np.ascontiguousarray(wv_full[:, s].astype(NPBF16)),
            "wo_t": np.ascontiguousarray(wo_full[s, :].astype(NPBF16)),
            "qb": np.ascontiguousarray(qb_full[s].reshape(4, P).T),
            "kb": np.ascontiguousarray(kb_full[s].reshape(4, P).T),
            "vb": np.ascontiguousarray(vb_full[s].reshape(1, FPC)),
        })
    in_maps = []
    for c in range(NCORES):
        b, j = c // 2, c % 2
        m = dict(jmaps[j])
        m["hid_s"] = np.ascontiguousarray(hid[b])
        m["kv_s"] = np.ascontiguousarray(kv[b])
        in_maps.append(m)
    return in_maps


def kernel(hidden_states, key_value_states, q_weight, q_bias,
           kv_weight, kv_bias, out_weight, out_bias, _trace=False):
    if "nc" not in _CACHE:
        _CACHE["nc"] = _build_core_program()
    nc = _CACHE["nc"]
    in_maps = _prep_inputs(hidden_states, key_value_states, q_weight, q_bias,
                           kv_weight, kv_bias, out_weight, out_bias)
    res = bass_utils.run_bass_kernel_spmd(
        nc, in_maps, core_ids=list(range(NCORES)), trace=_trace)
    _CACHE["last_result"] = res
    ob = np.asarray(out_bias, np.float32)
    out = np.empty((B, LQ, D), np.float32)
    for b in range(B):
        p0 = np.asarray(res.results[2 * b]["out_s"], np.float32)
        p1 = np.asarray(res.results[2 * b + 1]["out_s"], np.float32)
        out[b] = p0 + p1 + ob
    return out


# revision 27
# speedup vs baseline: 1.0092x; 1.0070x over previous
"""Trainium2 Bass kernel for nn_BartCrossAttention (B=4, L=1024, D=1024, H=16, HD=64).

v2 sharding: core c -> (batch b = c//2, head-half j = c%2). Each core computes
heads [8j, 8j+8) for ALL 1024 query tokens of its batch, including the K/V/Q
projections restricted to its 512 features, then a PARTIAL out-projection
(contracting only its 512 ctx features). The host sums the two partial outputs
per batch and adds out_bias. No KV-projection duplication, no collectives.

Per-core dataflow (bf16 on every PE input, fp32 PSUM accumulation):
  stage:  kv/hid -> SBUF (2 big DMAs each); weights via gpsimd-issued DMAs
  PE-transpose kv -> kvT [128,8,1024], hid -> hidT
  V = kvT.T @ Wv + vb  -> v65 [tok, head-blocks of 64 | ones col] (ones col
      gives softmax denominators for free in AV row 64)
  K^T = Wk.T @ kvT + kb -> KT [128,4,1024]; Q^T likewise (Wq pre-scaled 1/8)
  per head h (f = h//2, partitions rb=64*(h%2)): per t (kv tile), chunk c:
      S^T = KT_h.T @ qT_h (single 64-contraction matmul); attn = exp(S^T) on
      ACT (psum->bf16); ctx_ps[c] += [V_h|1].T @ attn
  normalization pipelined into the next head's slots: recip of sums row (DVE)
      -> ones-broadcast matmul (PE, 64x512) -> fused evict-multiply into ctxT
  out partial = ctxT.T @ Wo -> bf16 -> DRAM (bias added on host)
"""
import sys

for _p in ("/opt/trn_rl_repo",):
    if _p not in sys.path:
        sys.path.insert(0, _p)

import numpy as np
import ml_dtypes

import concourse.bass as bass
import concourse.mybir as mybir
import concourse.tile as tile
from concourse import bacc
import concourse.bass_utils as bass_utils
from concourse.masks import make_identity

F32 = mybir.dt.float32
F32R = mybir.dt.float32r
BF16 = mybir.dt.bfloat16
NPBF16 = ml_dtypes.bfloat16

P = 128
D = 1024        # model dim
H = 16          # heads (global)
HPC = 8         # heads per core
FPC = 512       # features per core
NCORES = 8
B, LQ, LK = 4, 1024, 1024

_CACHE = {}


def _build_core_program():
    nc = bacc.Bacc("TRN2", target_bir_lowering=False, debug=False,
                   num_devices=NCORES)

    hid_s = nc.dram_tensor("hid_s", [LQ, D], BF16, kind="ExternalInput")
    kv_s = nc.dram_tensor("kv_s", [LK, D], BF16, kind="ExternalInput")
    wq_t = nc.dram_tensor("wq_t", [D, FPC], BF16, kind="ExternalInput")
    wk_t = nc.dram_tensor("wk_t", [D, FPC], BF16, kind="ExternalInput")
    wv_t = nc.dram_tensor("wv_t", [D, FPC], BF16, kind="ExternalInput")
    wo_t = nc.dram_tensor("wo_t", [FPC, D], BF16, kind="ExternalInput")
    qb_d = nc.dram_tensor("qb", [P, 4], F32, kind="ExternalInput")
    kb_d = nc.dram_tensor("kb", [P, 4], F32, kind="ExternalInput")
    vb_d = nc.dram_tensor("vb", [1, FPC], F32, kind="ExternalInput")
    out_s = nc.dram_tensor("out_s", [LQ, D], BF16, kind="ExternalOutput")

    Exp = mybir.ActivationFunctionType.Exp
    Ident = mybir.ActivationFunctionType.Identity
    add = mybir.AluOpType.add
    mult = mybir.AluOpType.mult

    with tile.TileContext(nc) as tc:
        with (
            tc.tile_pool(name="setup", bufs=1) as setup,
            tc.tile_pool(name="big", bufs=1) as big,
        ):
            # ---- staging tiles + all input DMAs up front, spread across
            # issue engines so no single queue serializes the prologue ----
            wk = setup.tile([P, 8, FPC], BF16, tag="wk")
            wq = setup.tile([P, 8, FPC], BF16, tag="wq")
            wv = setup.tile([P, 8, FPC], BF16, tag="wv")
            wo = setup.tile([P, 4, D], BF16, tag="wo")

            # identity first: gpsimd builds it before issuing its DMAs so
            # the PE warm-up can start immediately
            identF = setup.tile([P, P], F32, tag="identF")
            make_identity(nc, identF[:])
            ident = setup.tile([P, P], BF16, tag="ident")
            nc.vector.tensor_copy(ident[:], identF[:])

            nc.gpsimd.dma_start(
                wv[:], wv_t.ap().rearrange("(dd p) o -> p dd o", p=P))
            nc.gpsimd.dma_start(
                wk[:], wk_t.ap().rearrange("(dd p) o -> p dd o", p=P))
            nc.gpsimd.dma_start(
                wq[:], wq_t.ap().rearrange("(dd p) o -> p dd o", p=P))
            nc.gpsimd.dma_start(
                wo[:], wo_t.ap().rearrange("(dd p) o -> p dd o", p=P))

            qb_sb = setup.tile([P, 4], F32, tag="qb")
            nc.gpsimd.dma_start(qb_sb[:], qb_d.ap())
            kb_sb = setup.tile([P, 4], F32, tag="kb")
            nc.gpsimd.dma_start(kb_sb[:], kb_d.ap())
            vb_row = setup.tile([1, FPC], F32, tag="vb_row")
            nc.gpsimd.dma_start(vb_row[:], vb_d.ap())

            # ---- small setup ----
            vbB = setup.tile([P, FPC], F32, tag="vbB")
            nc.gpsimd.partition_broadcast(vbB[:], vb_row[:])

            # ---- persistent big tiles ----
            kvT = big.tile([P, 8, LK], BF16, tag="kvT")    # kv^T [1024,1024]
            hidT = big.tile([P, 8, LQ], BF16, tag="hidT")  # hid^T
            # crossbar-transposing DMAs fill kvT/hidT directly from DRAM
            for f in range(8):
                nc.sync.dma_start_transpose(
                    kvT[:, f, :], kv_s.ap()[:, f * P:(f + 1) * P])
            for f in range(8):
                nc.scalar.dma_start_transpose(
                    hidT[:, f, :], hid_s.ap()[:, f * P:(f + 1) * P])
            KT = big.tile([P, 4, LK], BF16, tag="KT")      # K^T [512,1024]
            qT = big.tile([P, 4, LQ], BF16, tag="qT")      # Q^T [512,1024]
            v65 = big.tile([P, 8, HPC * 65], BF16, tag="v65")
            ctxT = big.tile([P, 4, LQ], BF16, tag="ctxT")  # ctx^T [512,1024]

            # ones column (col 64 of each head block) for denominators
            onesF = setup.tile([P, 64], F32, tag="onesF")
            nc.gpsimd.memset(onesF[:], 1.0)
            nc.vector.tensor_copy(
                v65[:].rearrange("p t (h x) -> p t h x", x=65)[:, :, :, 64:65],
                onesF[:].rearrange("p (t h x) -> p t h x", t=8, h=8))

            # ---- HAM warm-up: ~24 full-array matmuls on junk data while
            # the first kv chunks are still in flight. Triggers the PE
            # activity monitor to 8/8 (2.4 GHz) before real work arrives ----
            with tc.tile_pool(name="warm", bufs=1, space="PSUM") as warmp:
                wps = warmp.tile([P, P], F32, tag="warm")
                for i in range(40):
                    nc.tensor.matmul(wps[:], ident[:], ident[:],
                                     start=(i == 0), stop=(i == 39))


            if True:
                # ---- V projection: v65[:, ti, h*65:h*65+64] ----
                def emit_vproj(ti, pool):
                    pv = pool.tile([P, FPC], F32, tag="pp", name=f"pv{ti}")
                    for dd in range(8):
                        nc.tensor.matmul(
                            pv[:],
                            kvT[:, dd, ti * P:(ti + 1) * P],
                            wv[:, dd, :],
                            start=(dd == 0), stop=(dd == 7),
                        )
                    dst = v65[:].rearrange("p t (h x) -> p t h x", x=65)[
                        :, ti, :, 0:64]
                    nc.vector.tensor_tensor(dst, pv[:], vbB[:], add)

                # ---- K^T / Q^T projections (feature tile f, token chunk ck)
                def emit_kproj(f, ck, pool):
                    pk = pool.tile([P, FPC], F32, tag="pp", name=f"pk{f}_{ck}")
                    for dd in range(8):
                        nc.tensor.matmul(
                            pk[:],
                            wk[:, dd, f * P:(f + 1) * P],
                            kvT[:, dd, ck * 512:(ck + 1) * 512],
                            start=(dd == 0), stop=(dd == 7),
                        )
                    nc.vector.tensor_scalar(
                        KT[:, f, ck * 512:(ck + 1) * 512], pk[:],
                        kb_sb[:, f:f + 1], None, add)

                def emit_qproj(f, ck, pool):
                    pq = pool.tile([P, FPC], F32, tag="pp", name=f"pq{f}_{ck}")
                    for dd in range(8):
                        nc.tensor.matmul(
                            pq[:],
                            wq[:, dd, f * P:(f + 1) * P],
                            hidT[:, dd, ck * 512:(ck + 1) * 512],
                            start=(dd == 0), stop=(dd == 7),
                        )
                    nc.vector.tensor_scalar(
                        qT[:, f, ck * 512:(ck + 1) * 512], pq[:],
                        qb_sb[:, f:f + 1], None, add)

                with tc.tile_pool(name="psmmA", bufs=3,
                                  space="PSUM") as psmmA:
                    for ti in range(8):
                        emit_vproj(ti, psmmA)
                    for ck in range(2):
                        emit_kproj(0, ck, psmmA)
                    for ck in range(2):
                        emit_qproj(0, ck, psmmA)

                # ---- attention main loop ----
                with (
                    tc.tile_pool(name="scp", bufs=2, space="PSUM") as scp,
                    tc.tile_pool(name="ctxp", bufs=3, space="PSUM") as ctxp,
                    tc.tile_pool(name="psmmB", bufs=1, space="PSUM") as psmmB,
                    tc.tile_pool(name="bcbp", bufs=2) as bcbp,
                    tc.tile_pool(name="atp", bufs=3) as atp,
                    tc.tile_pool(name="rcpp", bufs=2) as rcpp,
                ):
                    def emit_norm_recips(h, ctx_pair):
                        # sums rows: psum -> sbuf stage, then fast reciprocal
                        rcps = []
                        for c in range(2):
                            stg = rcpp.tile([1, FPC], F32, tag=f"stg{c}",
                                            name=f"stg{h}_{c}")
                            nc.vector.tensor_copy(stg[:],
                                                  ctx_pair[c][64:65, :])
                            rcp = rcpp.tile([1, FPC], F32, tag=f"rcp{c}",
                                            name=f"rcp{h}_{c}")
                            nc.vector.reciprocal_approx_fast(rcp[:], stg[:])
                            rcps.append(rcp)
                        return rcps

                    def emit_norm_bc(h, rcps):
                        # broadcast each recip row to 64 partitions (gpsimd)
                        bcs = []
                        for c in range(2):
                            bcb = bcbp.tile([64, FPC], F32, tag=f"bcb{c}",
                                            name=f"bcb{h}_{c}")
                            nc.gpsimd.partition_broadcast(bcb[:], rcps[c][:])
                            bcs.append(bcb)
                        return bcs

                    def emit_norm_mult(h, ctx_pair, bcs):
                        f, rb = h // 2, 64 * (h % 2)
                        for c in range(2):
                            nc.vector.tensor_tensor(
                                ctxT[rb:rb + 64, f, c * 512:(c + 1) * 512],
                                ctx_pair[c][0:64, :], bcs[c][:], mult)

                    prev = None  # head pending normalization
                    for h in range(HPC):
                        f, rb = h // 2, 64 * (h % 2)
                        nxt_f = f + 1
                        ctx_pair = [ctxp.tile([65, FPC], F32, tag="ctx",
                                              name=f"ctx{h}_{c}")
                                    for c in range(2)]
                        at_prev = None
                        for t in range(8):
                            sc = scp.tile([P, 2 * FPC], F32, tag="sc",
                                          name=f"sc{h}_{t}")
                            for c in range(2):
                                nc.tensor.matmul(
                                    sc[:, c * FPC:(c + 1) * FPC],
                                    KT[rb:rb + 64, f, t * P:(t + 1) * P],
                                    qT[rb:rb + 64, f, c * 512:(c + 1) * 512],
                                    start=True, stop=True,
                                )
                            # pipelined normalization of the previous head
                            if prev is not None and t == 0:
                                ph, pctx = prev
                                rcps = emit_norm_recips(ph, pctx)
                                bcs = emit_norm_bc(ph, rcps)
                                emit_norm_mult(ph, pctx, bcs)
                                prev = None
                            # one 1024-wide exp per t (full ACT column rate)
                            at = atp.tile([P, 2 * FPC], BF16, tag="at",
                                          name=f"at{h}_{t}")
                            nc.scalar.activation(at[:], sc[:], Exp)
                            # AV lags one t: PE never waits on a fresh exp
                            if at_prev is not None:
                                for c in range(2):
                                    nc.tensor.matmul(
                                        ctx_pair[c][:],
                                        v65[:, t - 1, h * 65:(h + 1) * 65],
                                        at_prev[:, c * FPC:(c + 1) * FPC],
                                        start=(t == 1), stop=False,
                                    )
                            at_prev = at
                            # full-array projection filler: keeps the PE HAM
                            # activity monitor asserting warm (2.4 GHz) during
                            # the half-array attention matmuls
                            if nxt_f < 4:
                                if h % 2 == 0:
                                    if t == 2:
                                        emit_kproj(nxt_f, 0, psmmB)
                                    elif t == 5:
                                        emit_kproj(nxt_f, 1, psmmB)
                                else:
                                    if t == 2:
                                        emit_qproj(nxt_f, 0, psmmB)
                                    elif t == 5:
                                        emit_qproj(nxt_f, 1, psmmB)
                        for c in range(2):
                            nc.tensor.matmul(
                                ctx_pair[c][:],
                                v65[:, 7, h * 65:(h + 1) * 65],
                                at_prev[:, c * FPC:(c + 1) * FPC],
                                start=False, stop=True,
                            )
                        prev = (h, ctx_pair)
                    # final head's normalization
                    ph, pctx = prev
                    rcps = emit_norm_recips(ph, pctx)
                    bcs = emit_norm_bc(ph, rcps)
                    emit_norm_mult(ph, pctx, bcs)

            # ---- epilogue: partial out projection (no bias; host adds) ----
            with (
                tc.tile_pool(name="pop", bufs=2, space="PSUM") as pop,
                tc.tile_pool(name="outp", bufs=2) as outp,
            ):
                for m in range(8):
                    ot = outp.tile([P, D], BF16, tag="ot", name=f"ot{m}")
                    for half in range(2):
                        po = pop.tile([P, FPC], F32, tag="po",
                                      name=f"po{m}_{half}")
                        for fj in range(4):
                            nc.tensor.matmul(
                                po[:],
                                ctxT[:, fj, m * P:(m + 1) * P],
                                wo[:, fj, half * 512:(half + 1) * 512],
                                start=(fj == 0), stop=(fj == 3),
                            )
                        if half == 0:
                            nc.scalar.activation(
                                ot[:, half * 512:(half + 1) * 512], po[:],
                                Ident)
                        else:
                            nc.vector.tensor_copy(
                                ot[:, half * 512:(half + 1) * 512], po[:])
                    eng = nc.sync if m % 2 == 0 else nc.gpsimd
                    eng.dma_start(
                        out_s.ap().rearrange("(mm p) d -> p mm d", p=P)[
                            :, m, :],
                        ot[:])

    nc.compile()
    return nc


def _prep_inputs(hidden_states, key_value_states, q_weight, q_bias,
                 kv_weight, kv_bias, out_weight, out_bias):
    f32 = np.float32
    hid = np.asarray(hidden_states, f32).reshape(B, LQ, D).astype(NPBF16)
    kv = np.asarray(key_value_states, f32).reshape(B, LK, D).astype(NPBF16)
    scale = f32(1.0 / 8.0)

    # de-interleave kv rows: row e <-> (h=e//128, j=(e%128)//64, d=e%64)
    e = np.arange(2 * D)
    kmask = (e % 128) < 64
    kidx, vidx = e[kmask], e[~kmask]
    kvw = np.asarray(kv_weight, f32)
    kvb = np.asarray(kv_bias, f32)

    wq_full = (np.asarray(q_weight, f32) * scale).T      # [D, D] d x feat
    wk_full = kvw[kidx].T                                # [D, D]
    wv_full = kvw[vidx].T
    wo_full = np.asarray(out_weight, f32).T              # [D, D] feat x out
    qb_full = np.asarray(q_bias, f32) * scale
    kb_full = kvb[kidx]
    vb_full = kvb[vidx]

    jmaps = []
    for j in range(2):
        s = slice(j * FPC, (j + 1) * FPC)
        jmaps.append({
            "wq_t": np.ascontiguousarray(wq_full[:, s].astype(NPBF16)),
            "wk_t": np.ascontiguousarray(wk_full[:, s].astype(NPBF16)),
            "wv_t": np.ascontiguousarray(wv_full[:, s].astype(NPBF16)),
            "wo_t": np.ascontiguousarray(wo_full[s, :].astype(NPBF16)),
            "qb": np.ascontiguousarray(qb_full[s].reshape(4, P).T),
            "kb": np.ascontiguousarray(kb_full[s].reshape(4, P).T),
            "vb": np.ascontiguousarray(vb_full[s].reshape(1, FPC)),
        })
    in_maps = []
    for c in range(NCORES):
        b, j = c // 2, c % 2
        m = dict(jmaps[j])
        m["hid_s"] = np.ascontiguousarray(hid[b])
        m["kv_s"] = np.ascontiguousarray(kv[b])
        in_maps.append(m)
    return in_maps


def kernel(hidden_states, key_value_states, q_weight, q_bias,
           kv_weight, kv_bias, out_weight, out_bias, _trace=False):
    if "nc" not in _CACHE:
        _CACHE["nc"] = _build_core_program()
    nc = _CACHE["nc"]
    in_maps = _prep_inputs(hidden_states, key_value_states, q_weight, q_bias,
                           kv_weight, kv_bias, out_weight, out_bias)
    res = bass_utils.run_bass_kernel_spmd(
        nc, in_maps, core_ids=list(range(NCORES)), trace=_trace)
    _CACHE["last_result"] = res
    ob = np.asarray(out_bias, np.float32)
    out = np.empty((B, LQ, D), np.float32)
    for b in range(B):
        p0 = np.asarray(res.results[2 * b]["out_s"], np.float32)
        p1 = np.asarray(res.results[2 * b + 1]["out_s"], np.float32)
        out[b] = p0 + p1 + ob
    return out
